# revision 1
# baseline (speedup 1.0000x reference)
"""Transposed-layout kernel (v3): TensorE does all three reductions.

Host sends vT, gT [1000, 2048] bf16 per core (L on the row axis).  On
device, 8 chunks of 125 L-rows land on partitions; the per-ROW (per
output column) reductions n = 1'v, Z = 1'exp(g), svl = 1'(v*g) become
ones-stationary matmuls contracting over partitions, accumulated in
PSUM f32 across chunks.  A [125, 9] selector stationary routes each
reduction to its own partition row of shared [3, 512] PSUM banks
(4 strips of 512 columns), so all 3 reductions fit in 4 banks.

Engine budget per core: DMA 8.2 MB ~20us, ACT exp 8x2.0us, DVE tt
8x1.13us + 4 PSUM drains, TensorE 96 matmuls ~21us.  n is EXACT
(1.0*bf16 int accumulated in f32 PSUM).
"""

import math
import os

if os.environ.get("JAX_PLATFORMS", "") in ("cpu", "CPU"):
    os.environ.pop("JAX_PLATFORMS")

import ml_dtypes
import numpy as np

import concourse.bass as bass
import concourse.mybir as mybir
from concourse import bacc
from concourse.bass_utils import run_bass_kernel_spmd

B = 16384
L = 1000
LP = 1024  # L padded so every matmul contracts a full 128-row PE tile
N_CORES = 8
ROWS = B // N_CORES  # 2048 output columns per core
PCH = 128  # partitions per chunk
NCH = 8  # chunks (8 * 128 = 1024 = LP)
NSTRIP = 4
SW = ROWS // NSTRIP  # 512 columns per strip = one PSUM bank
WEIGHT_MSE = 1.0

_CACHE: dict = {}


def _build_module(detect_races: bool = True) -> bass.Bass:
    nc = bacc.Bacc(
        "TRN2",
        target_bir_lowering=False,
        debug=False,
        num_devices=N_CORES,
        detect_race_conditions=detect_races,
    )
    f32 = mybir.dt.float32
    bf16 = mybir.dt.bfloat16
    AF = mybir.ActivationFunctionType
    OP = mybir.AluOpType

    v_d = nc.dram_tensor("true_counts", [LP, ROWS], bf16, kind="ExternalInput").ap()
    g_d = nc.dram_tensor("logits", [LP, ROWS], bf16, kind="ExternalInput").ap()
    sel_d = nc.dram_tensor("sel", [PCH, 16], bf16, kind="ExternalInput").ap()
    st_d = nc.dram_tensor("stats", [3, NSTRIP, SW], f32, kind="ExternalOutput").ap()

    v_pr = v_d.rearrange("(c p) n -> p c n", p=PCH)
    g_pr = g_d.rearrange("(c p) n -> p c n", p=PCH)

    from contextlib import ExitStack

    with ExitStack() as ctx:
        e = ctx.enter_context
        vt = e(nc.sbuf_tensor([PCH, NCH, ROWS], bf16))
        gt = e(nc.sbuf_tensor([PCH, NCH, ROWS], bf16))
        et = e(nc.sbuf_tensor([PCH, NCH, ROWS], bf16))
        pt = e(nc.sbuf_tensor([PCH, NCH, ROWS], bf16))
        sel = e(nc.sbuf_tensor([PCH, 16], bf16))
        st_sb = e(nc.sbuf_tensor([3, NSTRIP, SW], f32))
        psum = [e(nc.psum_tensor(f"ps{i}", [3, SW], f32)) for i in range(NSTRIP)]
        dma_v = e(nc.semaphore("dma_v"))
        dma_g = e(nc.semaphore("dma_g"))
        act_done = e(nc.semaphore("act_done"))
        dma_sel = e(nc.semaphore("dma_sel"))
        et_done = e(nc.semaphore("et_done"))
        pt_done = e(nc.semaphore("pt_done"))
        mm_done = e(nc.semaphore("mm_done"))
        dve_done = e(nc.semaphore("dve_done"))
        out_done = e(nc.semaphore("out_done"))

        block = bass.BassBlock(nc, f"main{nc.next_id()}")
        block.__enter__()

        def sync_body(sync):
            sync.dma_start(sel[:], sel_d[:]).then_inc(dma_sel, 16)
            for c in range(NCH):
                sync.dma_start(vt[:, c, :], v_pr[:, c, :]).then_inc(dma_v, 16)
            sync.wait_ge(act_done, 1)
            sync.wait_ge(dve_done, 1)
            sync.dma_start(st_d[:], st_sb[:]).then_inc(out_done, 16)
            sync.wait_ge(out_done, 16)

        def scalar_body(scalar):
            posted = [0]

            def post():
                if posted[0] < NCH:
                    c = posted[0]
                    scalar.dma_start(gt[:, c, :], g_pr[:, c, :]).then_inc(
                        dma_g, 16
                    )
                    posted[0] += 1

            post()
            post()
            # warmup: trigger exp table load while DMA streams
            scalar.activation(et[:, 0, 0:1], et[:, 0, 0:1], AF.Exp, scale=0.0)
            for c in range(NCH):
                scalar.wait_ge(dma_g, 16 * (c + 1))
                post()
                scalar.activation(et[:, c, :], gt[:, c, :], AF.Exp).then_inc(
                    et_done, 1
                )
            scalar.activation(
                et[0:1, 0, 0:1], et[0:1, 0, 0:1], AF.Exp, scale=0.0
            ).then_inc(act_done, 1)

        def vector_body(vector):
            for c in range(NCH):
                vector.wait_ge(dma_v, 16 * (c + 1))
                vector.wait_ge(dma_g, 16 * (c + 1))
                vector.tensor_tensor(
                    pt[:, c, :], vt[:, c, :], gt[:, c, :], OP.mult
                ).then_inc(pt_done, 1)
            vector.wait_ge(mm_done, 1)
            for st_i in range(NSTRIP):
                vector.tensor_copy(st_sb[:, st_i, :], psum[st_i][:])
            # fence: read last drained strip before releasing the store
            vector.tensor_copy(sel[0:1, 0:1], st_sb[0:1, NSTRIP - 1, 0:1]).then_inc(
                dve_done, 1
            )

        def tensor_body(tensor):
            # consume et/pt one chunk behind their producers: the ~2.4us of
            # lag guarantees the producer's SBUF writes have committed (an
            # immediate read after the sem inc races the write ack on HW)
            def pe_mms(tensor, c, stop_last=False):
                tensor.wait_ge(pt_done, c + 1)
                for s in range(NSTRIP):
                    tensor.matmul(
                        psum[s][:],
                        sel[:, 8:11],
                        pt[:, c, s * SW : (s + 1) * SW],
                        start=False,
                        stop=False,
                        skip_group_check=True,
                    )
                tensor.wait_ge(et_done, c + 1)
                last = None
                for s in range(NSTRIP):
                    last = tensor.matmul(
                        psum[s][:],
                        sel[:, 4:7],
                        et[:, c, s * SW : (s + 1) * SW],
                        start=False,
                        stop=stop_last,
                        skip_group_check=True,
                    )
                return last

            tensor.wait_ge(dma_sel, 16)
            for c in range(NCH):
                tensor.wait_ge(dma_v, 16 * (c + 1))
                for s in range(NSTRIP):
                    tensor.matmul(
                        psum[s][:],
                        sel[:, 0:3],
                        vt[:, c, s * SW : (s + 1) * SW],
                        start=(c == 0),
                        stop=False,
                        skip_group_check=True,
                    )
                if c >= 1:
                    pe_mms(tensor, c - 1)
            last = pe_mms(tensor, NCH - 1, stop_last=True)
            last.then_inc(mm_done, 1)

        block.sync(sync_body)
        block.scalar(scalar_body)
        block.vector(vector_body)
        block.tensor(tensor_body)

        # manual Block exit WITHOUT the all-engine butterfly barrier
        for engine, last_body in block.last_body.items():
            with nc.body(last_body, parent=nc.cur_bb, allow_existing_parent=True):
                engine.br(block.end_bb)
        nc.switch_bb(block.end_bb)

    nc.compile()
    return nc


def _get_module() -> bass.Bass:
    if "nc" not in _CACHE:
        _CACHE["nc"] = _build_module()
    return _CACHE["nc"]


def _run_device(true_counts: np.ndarray, logits: np.ndarray, **kwargs):
    nc = _get_module()
    bf16 = ml_dtypes.bfloat16
    v = np.ascontiguousarray(true_counts, dtype=np.float32).astype(bf16)
    g = np.ascontiguousarray(logits, dtype=np.float32).astype(bf16)

    def padT(a, fill):
        out = np.full((LP, a.shape[0]), fill, dtype=bf16)
        out[:L] = a.T
        return out

    sel_np = np.zeros((PCH, 16), dtype=bf16)
    sel_np[:, 0] = 1.0   # n   -> psum row 0
    sel_np[:, 5] = 1.0   # Z   -> psum row 1
    sel_np[:, 10] = 1.0  # svl -> psum row 2
    in_maps = [
        {
            "true_counts": padT(v[c * ROWS : (c + 1) * ROWS], 0.0),
            "logits": padT(g[c * ROWS : (c + 1) * ROWS], -30.0),
            "sel": sel_np,
        }
        for c in range(N_CORES)
    ]
    res = run_bass_kernel_spmd(nc, in_maps, core_ids=list(range(N_CORES)), **kwargs)
    return [res.results[c]["stats"] for c in range(N_CORES)], res


def _host_combine(
    stats_per_core, true_counts: np.ndarray, tot_pred: np.ndarray
) -> np.ndarray:
    # exact global sum of lgamma(v+1) via histogram (v is integer 0..10)
    vi = np.asarray(true_counts, dtype=np.uint8)
    cnt = np.bincount(vi.reshape(-1), minlength=32)
    lg_table = np.array([math.lgamma(k + 1.0) for k in range(len(cnt))])
    s_lg = float(cnt @ lg_table)

    n_all = []
    lp_sum = -s_lg
    for s in stats_per_core:
        s = s.astype(np.float64)
        n = s[0].reshape(-1)  # column s*512+i = shard row index
        Z = s[1].reshape(-1)
        svl = s[2].reshape(-1)
        n_all.append(n)
        lgn = np.array([math.lgamma(x + 1.0) for x in n])
        lp_sum += lgn.sum() + svl.sum() - (n * np.log(Z)).sum()
    n_all = np.concatenate(n_all)
    mnlll = -lp_sum / B
    mse = np.mean((n_all - tot_pred.astype(np.float64).reshape(-1)) ** 2)
    return np.float32(WEIGHT_MSE * mse + mnlll)


def kernel(true_counts: np.ndarray, logits: np.ndarray, tot_pred: np.ndarray):
    stats, _ = _run_device(true_counts, logits)
    return _host_combine(stats, true_counts, tot_pred)

